# revision 9
# baseline (speedup 1.0000x reference)
"""Trainium2 Bass kernel for nn_Decoder (LSTM decoder with SE/HP MLP heads).

Strategy: pure data parallelism over batch (2048 -> 8 cores x 256).
Feature-major on-chip layout ([feature, batch]); weights stationary, batch
on the matmul moving dim. The SE MLP's output projection is folded into the
gate weights on the host (w2t = se_w2 @ w_ih.T), shrinking the x-part
contraction from K=512 to K=16, and all biases are folded into extra fp8
contraction rows (hi/lo split) so the gate bias is free.

fp8 version: the h-part gate matmuls (K=512) run as e4m3 DoubleRow
(double-pumped) matmuls at 2x bf16 throughput: per 128-row output tile,
2 DR instructions (2 k-tile groups each) + 1 plain fp8 x-part close.
Weights are scaled x16 into e4m3 (avoids subnormals), moving operands
(h, u, lp, r) unscaled / x16; activations descale via the ACT scale
operand. Gate order is (i, f, o, g). ReLUs run on DVE (tensor_scalar
max), gate activations emit bf16 so the LSTM cell DVE ops hit the
2x/4x modes; the lp recurrence carry stays f32 (x256) for accuracy.

Per step (32 sequential steps):
  u16 = max(A1_8.T @ lp8_ext, 0)                  [16, 256]  (fp8 MM + DVE)
  gates = DR(whh8, h8) + w2tx.T @ u_ext           [2048,256] (fp8, PSUM=16x)
  i,f,o = sigmoid(g/16), g = tanh(g/16)           (ACT, bf16 out)
  c = f*c + i*g; h8 = o*tanh(c)                   (DVE f32/bf16, h -> fp8)
  r16 = max(DR(b1_8, h8) + 16*c1hp, 0)            (fp8 DR + DVE)
  p256 = hpw2_8.T @ r16; s = p256 + 256*lp        (fp8 MM + DVE)
  lp = sigmoid(s/256 + hpb2) -> traj[t] (f32); lp8, lp256 via DVE
"""

import json

import numpy as np
import ml_dtypes
from contextlib import ExitStack

import concourse.bass as bass
import concourse.mybir as mybir
import concourse.tile as tile
from concourse.bass import ts


def _fix_multiwait(bir_bytes: bytes) -> bytes:
    """Hoist excess sync waits onto injected EventSemaphore carriers
    (HW cap: 2 waits on EventSemaphore, 1 elsewhere; the Tile end-of-kernel
    drain can exceed this and the compiler rejects it)."""
    bir = json.loads(bir_bytes)
    for fn in bir.get("functions", []):
        for blk in fn.get("blocks", []):
            insts = blk.get("instructions")
            if not insts:
                continue
            out = []
            for inst in insts:
                si = inst.get("sync_info")
                waits = (si or {}).get("on_wait") or []
                cap = 2 if inst.get("opcode") == "EventSemaphore" else 1
                if len(waits) > cap:
                    excess, keep = waits[:-cap], waits[-cap:]
                    si["on_wait"] = keep
                    for i in range(0, len(excess), 2):
                        out.append({
                            "debug": inst.get("debug", 0),
                            "engine": inst["engine"],
                            "ins": [],
                            "name": f"{inst['name']}_xw{i}",
                            "opcode": "EventSemaphore",
                            "outs": [],
                            "sync_info": {"on_update": [], "on_wait": excess[i : i + 2]},
                        })
                out.append(inst)
            blk["instructions"] = out
    return json.dumps(bir).encode()

BF16 = ml_dtypes.bfloat16
F8 = ml_dtypes.float8_e4m3
F32 = np.float32

SEQ = 32
B = 2048
H = 512
E = 512
HID = 16
NCORES = 8
BL = B // NCORES  # 256 local batch
NG = 4 * H  # 2048 gate features
KX = HID + 2  # x-part contraction: 16 u rows + bias hi/lo
BN_EPS = 1e-5

_CACHE: dict = {}


def _build_nc(repeats: int = 1):
    nc = bass.Bass()
    dt = mybir.dt
    ACTF = mybir.ActivationFunctionType
    ALU = mybir.AluOpType
    DR = mybir.MatmulPerfMode.DoubleRow

    # --- DRAM tensors (per-core inputs; weights replicated across cores) ---
    whh8_d = nc.dram_tensor("whh8", [2, 128, 2 * NG], dt.float8e4, kind="ExternalInput")
    w2tx_d = nc.dram_tensor("w2tx", [KX, NG], dt.float8e4, kind="ExternalInput")
    b18_d = nc.dram_tensor("b18", [2, 128, 2 * HID], dt.float8e4, kind="ExternalInput")
    a18_d = nc.dram_tensor("a18", [4, HID], dt.float8e4, kind="ExternalInput")
    hpw28_d = nc.dram_tensor("hpw28", [HID, 2], dt.float8e4, kind="ExternalInput")
    c1hp16_d = nc.dram_tensor("c1hp16", [HID, 1], dt.float32, kind="ExternalInput")
    hpb2_d = nc.dram_tensor("hpb2", [2, 1], dt.float32, kind="ExternalInput")
    uc_d = nc.dram_tensor("uc", [2, BL], dt.float8e4, kind="ExternalInput")
    lp8i_d = nc.dram_tensor("lp8i", [4, BL], dt.float8e4, kind="ExternalInput")
    lp256i_d = nc.dram_tensor("lp256i", [2, BL], dt.float32, kind="ExternalInput")
    h08_d = nc.dram_tensor("h08", [2, 128, 512], dt.float8e4, kind="ExternalInput")
    c0_d = nc.dram_tensor("c0", [2, 128, 512], dt.float32, kind="ExternalInput")
    traj_d = nc.dram_tensor("traj", [2, SEQ, BL], dt.float32, kind="ExternalOutput")

    with tile.TileContext(nc) as tc:
        with ExitStack() as ctx:
            singles = ctx.enter_context(tc.tile_pool(name="singles", bufs=1))
            gpool = ctx.enter_context(tc.tile_pool(name="gates", bufs=2))
            tpool = ctx.enter_context(tc.tile_pool(name="temps", bufs=3))
            psg = ctx.enter_context(tc.tile_pool(name="psg", bufs=7, space="PSUM"))
            pss = ctx.enter_context(tc.tile_pool(name="pss", bufs=1, space="PSUM"))

            # persistent weights
            whh8 = []
            for j in range(2):
                wt = singles.tile([128, 2 * NG], dt.float8e4, tag=f"whh8{j}", name=f"whh8{j}")
                nc.sync.dma_start(out=wt, in_=whh8_d[j, :, :])
                whh8.append(wt[:, :].rearrange("k (two m) -> k two m", two=2))
            w2tx = singles.tile([KX, NG], dt.float8e4, tag="w2tx", name="w2tx")
            nc.sync.dma_start(out=w2tx, in_=w2tx_d[:, :])
            b18 = []
            for j in range(2):
                bt = singles.tile([128, 2 * HID], dt.float8e4, tag=f"b18{j}", name=f"b18{j}")
                nc.sync.dma_start(out=bt, in_=b18_d[j, :, :])
                b18.append(bt[:, :].rearrange("k (two m) -> k two m", two=2))
            a18 = singles.tile([4, HID], dt.float8e4, tag="a18", name="a18")
            nc.sync.dma_start(out=a18, in_=a18_d[:, :])
            hpw28 = singles.tile([HID, 2], dt.float8e4, tag="hpw28", name="hpw28")
            nc.sync.dma_start(out=hpw28, in_=hpw28_d[:, :])
            c1hp16 = singles.tile([HID, 1], dt.float32, tag="c1hp16", name="c1hp16")
            nc.sync.dma_start(out=c1hp16, in_=c1hp16_d[:, :])
            hpb2 = singles.tile([2, 1], dt.float32, tag="hpb2", name="hpb2")
            nc.sync.dma_start(out=hpb2, in_=hpb2_d[:, :])

            # persistent state
            u_ext = singles.tile([KX, BL], dt.float8e4, tag="u_ext", name="u_ext")
            nc.sync.dma_start(out=u_ext[HID : HID + 2, :], in_=uc_d[:, :])
            lp8_ext = singles.tile([4, BL], dt.float8e4, tag="lp8", name="lp8")
            nc.sync.dma_start(out=lp8_ext, in_=lp8i_d[:, :])
            lp256 = singles.tile([2, BL], dt.float32, tag="lp256", name="lp256")
            nc.sync.dma_start(out=lp256, in_=lp256i_d[:, :])
            traj = singles.tile([2, SEQ * BL], dt.float32, tag="traj", name="traj")
            # h is ping-pong buffered: step t reads hb2[t%2], writes hb2[(t+1)%2]
            hb2, cst = {0: [], 1: []}, []
            for j in range(2):
                t_b = singles.tile([128, 512], dt.float8e4, tag=f"hbA{j}", name=f"hbA{j}")
                nc.sync.dma_start(out=t_b, in_=h08_d[j, :, :])
                hb2[0].append(t_b)
                t_b2 = singles.tile([128, 512], dt.float8e4, tag=f"hbB{j}", name=f"hbB{j}")
                hb2[1].append(t_b2)
                t_c = singles.tile([128, 512], dt.float32, tag=f"c{j}", name=f"c{j}")
                nc.sync.dma_start(out=t_c, in_=c0_d[j, :, :])
                cst.append(t_c)

            pairs: dict = {}

            def hview(t, jj):
                return hb2[t % 2][jj][:, :].rearrange("k (two b) -> k two b", two=2)

            def open_half0(t, p, jjs=(0, 1)):
                """Emit h-dependent DR matmuls for pair p's HALF 0 only — at
                most one open accumulation group per PSUM zero-region."""
                ps = pairs.get((t, p))
                if ps is None:
                    ps = psg.tile([128, 2 * BL], dt.float32, tag="gp", name=f"gp{t}_{p}")
                    pairs[(t, p)] = ps
                for jj in jjs:
                    nc.tensor.matmul(
                        ps[:, ts(0, BL)],
                        whh8[jj][:, :, ts(2 * p, 128)],
                        hview(t, jj),
                        start=(jj == 0), stop=False,
                        perf_mode=DR,
                    )
                return ps

            def xclose(t, p):
                """Close half 0 with the SE x-part (K=18, bias folded in),
                run half 1 as a complete sequential group, then evacuate via
                fused sigmoid/tanh (scale=1/16) to bf16."""
                ps = pairs.pop((t, p))
                nc.tensor.matmul(
                    ps[:, ts(0, BL)], w2tx[:, ts(2 * p, 128)], u_ext,
                    start=False, stop=True,
                )
                m = 2 * p + 1
                for jj in range(2):
                    nc.tensor.matmul(
                        ps[:, ts(1, BL)],
                        whh8[jj][:, :, ts(m, 128)],
                        hview(t, jj),
                        start=(jj == 0), stop=False,
                        perf_mode=DR,
                    )
                nc.tensor.matmul(
                    ps[:, ts(1, BL)], w2tx[:, ts(m, 128)], u_ext,
                    start=False, stop=True,
                )
                func = ACTF.Tanh if p in (6, 7) else ACTF.Sigmoid
                gs = gpool.tile(
                    [128, 2 * BL], dt.bfloat16, tag=f"gate{p}", name=f"gate{t}_{p}"
                )
                nc.scalar.activation(gs, ps, func, scale=1.0 / 16.0)
                return gs

            def elem_pre(t, j, gs):
                """LSTM cell c-update for feature-half j (DVE)."""
                i_t, f_t, g_t = gs[0 + j], gs[2 + j], gs[6 + j]
                t_ig = tpool.tile([128, 2 * BL], dt.bfloat16, tag="tig", name=f"tig{t}_{j}")
                t_fc = tpool.tile([128, 2 * BL], dt.float32, tag="tfc", name=f"tfc{t}_{j}")
                nc.vector.tensor_mul(t_ig, i_t, g_t)
                nc.vector.tensor_mul(t_fc, f_t, cst[j])
                nc.vector.tensor_add(cst[j], t_fc, t_ig)

            def elem_post(t, j, gs):
                """tanh(c) (ACT) and fp8 h write for feature-half j."""
                o_t = gs[4 + j]
                t_tc = tpool.tile([128, 2 * BL], dt.bfloat16, tag="ttc", name=f"ttc{t}_{j}")
                nc.scalar.activation(t_tc, cst[j], ACTF.Tanh)
                nc.vector.tensor_mul(hb2[(t + 1) % 2][j], o_t, t_tc)

            def body():
                # step-0 prologue (normally done in the previous step's tail)
                u_ps = pss.tile([HID, BL], dt.float32, tag="small", name="u_0")
                nc.tensor.matmul(u_ps, a18, lp8_ext, start=True, stop=True)
                for p in (0, 2, 4, 6, 1, 3):
                    open_half0(0, p)
                nc.vector.tensor_scalar(
                    u_ext[0:HID, :], u_ps, 0.0, None, op0=ALU.max
                )

                for t in range(SEQ):
                    nxt = t + 1 if t + 1 < SEQ else None
                    gs = {}
                    # j=0 pairs first so its elementwise chain overlaps j=1 MMs
                    for p in (0, 2, 4, 6):
                        gs[p] = xclose(t, p)
                    elem_pre(t, 0, gs)
                    for p in (1, 3):
                        gs[p] = xclose(t, p)
                    elem_post(t, 0, gs)
                    for p in (5, 7):
                        open_half0(t, p)
                        gs[p] = xclose(t, p)
                    elem_pre(t, 1, gs)
                    elem_post(t, 1, gs)

                    # tail: HP head chain interleaved with next-step fill waves
                    nh = hb2[(t + 1) % 2]

                    def nhview(jj):
                        return nh[jj][:, :].rearrange("k (two b) -> k two b", two=2)

                    v_ps = pss.tile([HID, BL], dt.float32, tag="small", name=f"v{t}")
                    nc.tensor.matmul(v_ps, b18[0], nhview(0), start=True, stop=False,
                                     perf_mode=DR)
                    if nxt is not None:
                        for p in (0, 2, 4, 6, 1, 3):
                            open_half0(nxt, p, jjs=(0,))
                    nc.tensor.matmul(v_ps, b18[1], nhview(1), start=False, stop=True,
                                     perf_mode=DR)
                    r16 = tpool.tile([HID, BL], dt.float8e4, tag="rhp", name=f"rhp{t}")
                    nc.vector.tensor_scalar(
                        r16, v_ps, c1hp16[:, :], 0.0, op0=ALU.add, op1=ALU.max
                    )
                    if nxt is not None:
                        open_half0(nxt, 0, jjs=(1,))
                        open_half0(nxt, 2, jjs=(1,))
                    p_ps = pss.tile([HID, BL], dt.float32, tag="small", name=f"p{t}")
                    nc.tensor.matmul(p_ps[:2, :], hpw28, r16, start=True, stop=True)
                    if nxt is not None:
                        open_half0(nxt, 4, jjs=(1,))
                        open_half0(nxt, 6, jjs=(1,))
                    # lp carry in f32 (x256) on DVE, then fused sigmoid
                    s_t = tpool.tile([2, BL], dt.float32, tag="st", name=f"st{t}")
                    nc.vector.tensor_add(s_t, p_ps[:2, :], lp256)
                    nc.scalar.activation(
                        traj[:2, ts(t, BL)], s_t, ACTF.Sigmoid,
                        bias=hpb2, scale=1.0 / 256.0,
                    )
                    nc.vector.tensor_scalar_mul(lp256, traj[:2, ts(t, BL)], 256.0)
                    nc.vector.tensor_copy(lp8_ext[0:2, :], traj[:2, ts(t, BL)])
                    if nxt is not None:
                        u_ps = pss.tile([HID, BL], dt.float32, tag="small", name=f"u{nxt}")
                        nc.tensor.matmul(u_ps, a18, lp8_ext, start=True, stop=True)
                        open_half0(nxt, 1, jjs=(1,))
                        open_half0(nxt, 3, jjs=(1,))
                        nc.vector.tensor_scalar(
                            u_ext[0:HID, :], u_ps, 0.0, None, op0=ALU.max
                        )

            if repeats == 1:
                body()
            else:
                with tc.For_i(0, repeats, 1):
                    body()

            nc.sync.dma_start(
                out=traj_d[:, :, :].rearrange("p t b -> p (t b)"), in_=traj[:2, :]
            )
    patched = _fix_multiwait(nc.to_json_bytes())
    nc.to_json_bytes = lambda: patched
    return nc


def _pack_half(x_t):
    # [512, BL] feature-major -> [2, 128, 2*BL]: tile j holds feature-tiles
    # 2j (cols 0:BL) and 2j+1 (cols BL:2BL)
    xr = x_t.reshape(4, 128, BL)
    return np.stack(
        [np.concatenate([xr[2 * j], xr[2 * j + 1]], axis=1) for j in range(2)]
    )


# gate-row permutation: reference order (i, f, g, o) -> kernel order (i, f, o, g)
_PERM = np.concatenate([np.arange(0, 1024), np.arange(1536, 2048), np.arange(1024, 1536)])


def _q8(x):
    return np.asarray(x, dtype=np.float64).astype(F8)


def _host_prep(inputs):
    f = lambda k: np.asarray(inputs[k], dtype=np.float64)
    se_w1, se_b1 = f("se_w1"), f("se_b1")
    se_g, se_bt, se_m, se_v = f("se_g"), f("se_bt"), f("se_m"), f("se_v")
    se_w2, se_b2 = f("se_w2"), f("se_b2")
    w_ih, w_hh, b_ih, b_hh = f("w_ih"), f("w_hh"), f("b_ih"), f("b_hh")
    hp_w1, hp_b1 = f("hp_w1"), f("hp_b1")
    hp_g, hp_bt, hp_m, hp_v = f("hp_g"), f("hp_bt"), f("hp_m"), f("hp_v")
    hp_w2, hp_b2 = f("hp_w2"), f("hp_b2")

    s_se = se_g / np.sqrt(se_v + BN_EPS)
    a1 = se_w1 * s_se[None, :]  # [2, 16]
    c1_se = (se_b1 - se_m) * s_se + se_bt  # [16]
    s_hp = hp_g / np.sqrt(hp_v + BN_EPS)
    b1 = hp_w1 * s_hp[None, :]  # [512, 16]
    c1_hp = (hp_b1 - hp_m) * s_hp + hp_bt  # [16]

    w2t = (se_w2 @ w_ih.T)[:, _PERM]  # [16, 2048]
    b_eff = (b_ih + b_hh + w_ih @ se_b2)[_PERM]  # [2048]
    w_hh_p = w_hh[_PERM, :]  # [2048, 512]

    # whh8 [2, 128, 2, NG]: [j, k, i, m] = e4m3(16*w_hh_p[m, (2j+i)*128+k])
    wT = (16.0 * w_hh_p.T).reshape(4, 128, NG)  # [kk, k, m]
    whh8 = np.stack(
        [np.stack([wT[2 * j + i] for i in range(2)], axis=1) for j in range(2)]
    )  # [2, 128, 2, NG]

    b_hi = _q8(b_eff)
    b_lo = _q8(16.0 * (b_eff - b_hi.astype(np.float64)))
    w2tx = np.concatenate(
        [_q8(w2t), b_hi[None, :], b_lo[None, :]], axis=0
    )  # [KX, NG] e4m3

    bT = (16.0 * b1).reshape(4, 128, HID)  # [kk, k, c]
    b18 = np.stack(
        [np.stack([bT[2 * j + i] for i in range(2)], axis=1) for j in range(2)]
    )  # [2, 128, 2, HID]

    c_hi = _q8(16.0 * c1_se)
    c_lo = _q8(16.0 * c1_se - c_hi.astype(np.float64))
    a18 = np.concatenate([_q8(16.0 * a1), c_hi[None, :], c_lo[None, :]], axis=0)

    rep = {
        "whh8": np.ascontiguousarray(_q8(whh8)).reshape(2, 128, 2 * NG),
        "w2tx": np.ascontiguousarray(w2tx),
        "b18": np.ascontiguousarray(_q8(b18)).reshape(2, 128, 2 * HID),
        "a18": np.ascontiguousarray(a18),
        "hpw28": np.ascontiguousarray(_q8(16.0 * hp_w2)),
        "c1hp16": (16.0 * c1_hp).astype(F32).reshape(HID, 1),
        "hpb2": hp_b2.astype(F32).reshape(2, 1),
    }

    last_pos = np.asarray(inputs["last_pos"], dtype=np.float64)
    h0 = np.asarray(inputs["hh"], dtype=np.float64)[0]
    c0 = np.asarray(inputs["ch"], dtype=np.float64)[0]
    in_maps = []
    for c in range(NCORES):
        rows = slice(c * BL, (c + 1) * BL)
        h0t = np.ascontiguousarray(h0[rows].T)  # [512, BL]
        c0t = np.ascontiguousarray(c0[rows].T)
        m = dict(rep)
        lp0t = np.ascontiguousarray(last_pos[rows].T)  # [2, BL]
        lp8i = np.empty((4, BL), dtype=F8)
        lp8i[0:2] = _q8(lp0t)
        lp8i[2:4] = F8(1.0)
        m["lp8i"] = lp8i
        uc = np.empty((2, BL), dtype=F8)
        uc[0] = F8(16.0)
        uc[1] = F8(1.0)
        m["uc"] = uc
        m["lp256i"] = (256.0 * lp0t).astype(F32)
        m["h08"] = _q8(_pack_half(h0t))
        m["c0"] = _pack_half(c0t).astype(F32)
        in_maps.append(m)
    return in_maps


def _get_runner(repeats: int = 1):
    """Build (once) a persistent jitted SPMD runner over 8 cores."""
    key = ("runner", repeats)
    if key in _CACHE:
        return _CACHE[key]

    import jax
    from jax.sharding import Mesh, PartitionSpec, NamedSharding
    from jax.experimental.shard_map import shard_map
    from concourse import bass2jax, mybir as _mb

    nc = _build_nc(repeats)
    bass2jax.install_neuronx_cc_hook()

    partition_name = nc.partition_id_tensor.name if nc.partition_id_tensor else None
    in_names, out_names, out_avals, zero_shapes = [], [], [], []
    for alloc in nc.m.functions[0].allocations:
        if not isinstance(alloc, _mb.MemoryLocationSet):
            continue
        name = alloc.memorylocations[0].name
        if alloc.kind == "ExternalInput":
            if name != partition_name:
                in_names.append(name)
        elif alloc.kind == "ExternalOutput":
            out_names.append(name)
            shape = tuple(alloc.tensor_shape)
            dtype = _mb.dt.np(alloc.dtype)
            out_avals.append(jax.core.ShapedArray(shape, dtype))
            zero_shapes.append((shape, dtype))
    n_params = len(in_names)
    all_names = in_names + out_names
    if partition_name is not None:
        all_names = all_names + [partition_name]
    donate = tuple(range(n_params, n_params + len(out_names)))

    def _body(*args):
        operands = list(args)
        if partition_name is not None:
            operands.append(bass2jax.partition_id_tensor())
        outs = bass2jax._bass_exec_p.bind(
            *operands,
            out_avals=tuple(out_avals),
            in_names=tuple(all_names),
            out_names=tuple(out_names),
            lowering_input_output_aliases=(),
            sim_require_finite=True,
            sim_require_nnan=True,
            nc=nc,
        )
        return tuple(outs)

    devices = jax.devices()[:NCORES]
    mesh = Mesh(np.asarray(devices), ("core",))
    spec = PartitionSpec("core")
    sharded = jax.jit(
        shard_map(
            _body,
            mesh=mesh,
            in_specs=(spec,) * (n_params + len(out_names)),
            out_specs=(spec,) * len(out_names),
            check_rep=False,
        ),
        donate_argnums=donate,
        keep_unused=True,
    )
    sharding = NamedSharding(mesh, spec)

    def stage(in_maps):
        """device_put concatenated inputs once; reusable across exec() calls."""
        concat = [
            np.concatenate([np.asarray(m[name]) for m in in_maps], axis=0)
            for name in in_names
        ]
        return [jax.device_put(a, sharding) for a in concat]

    def exec_(staged):
        zeros = [
            jax.device_put(np.zeros((NCORES * s[0], *s[1:]), d), sharding)
            for s, d in zero_shapes
        ]
        outs = sharded(*staged, *zeros)
        outs = [np.asarray(o) for o in outs]
        return {
            name: outs[i].reshape(NCORES, *out_avals[i].shape)
            for i, name in enumerate(out_names)
        }

    _CACHE[key] = (stage, exec_)
    return _CACHE[key]


def kernel(**inputs) -> np.ndarray:
    stage, exec_ = _get_runner()
    staged = stage(_host_prep(inputs))
    per_core = exec_(staged)["traj"]  # [8, 2, 32, BL]
    out = per_core.transpose(2, 0, 3, 1).reshape(SEQ, B, 2)
    return np.ascontiguousarray(out.astype(np.float32))
